# revision 17
# baseline (speedup 1.0000x reference)
"""Bottom-up FPN (3x3 conv + 3 deformable-conv levels) on 8 trn2 NeuronCores.

Sharding: batch(2) x row-quarters(4) -> 8 cores (core c: n=c//4, q=c%4).
Level 0: plain 3x3 conv via f32r matmuls (shifted-window rhs APs).
Levels 1-3: bum = x_l + maxpool2(out_{l-1}); 3x3 offset conv (18ch);
deformable 3x3 conv: bilinear sampling via dma_gather of 2px-wide
channel-last bf16 elements, corner-combine on DVE in the position-major
(transposed) domain with per-position scalars, PE transposes back to
channel-major, then 18-accumulation f32r matmuls per output tile.
Cross-core row halos move through AllGather collectives (groups of 4,
one group per batch element); per-core boundary masks select the right
neighbor blocks (SPMD-safe).
"""
import numpy as np
import ml_dtypes

import concourse.bass as bass
import concourse.bacc as bacc
import concourse.mybir as mybir
import concourse.tile as tile
from concourse import library_config

f32 = mybir.dt.float32
f32r = mybir.dt.float32r
bf16 = mybir.dt.bfloat16
i16 = mybir.dt.int16
i32 = mybir.dt.int32
AT = mybir.AluOpType

N_CORES = 8
C = 256
NCT = 2  # channel tiles


def level_params(H0):
    levels = []
    for li in range(1, 4):
        H = H0 >> li
        W = H
        R = H // 4
        P = 4              # row halo (covers |offset| < 3 bilinear reach)
        if P >= R:
            P = 2 * R      # full-AllGather path: two R-row blocks per side
        PX = 4
        Nidx = R * W
        Nidx_pad = max(Nidx, 128)
        Nt = Nidx_pad // 128
        levels.append(dict(
            li=li, H=H, W=W, R=R, P=P, PX=PX, Nidx=Nidx, Nidx_pad=Nidx_pad,
            Nt=Nt, Rg=R + 2 * P, Wg=W + 2 * PX, full_ag=(P >= R),
            NPX=(R + 2 * P) * (W + 2 * PX), NK=Nt * 9,
        ))
    return levels


# ------------------------------------------------------------------ device

def build_nc(H0=128, nlevels=3, use_coll=True, use_gather=True):
    LV = level_params(H0)[:nlevels]
    R0, W0 = H0 // 4, H0
    W0p = W0 + 2
    X0F = (R0 + 2) * W0p

    nc = bacc.Bacc("TRN2", num_devices=N_CORES)

    def inp(name, shape, dt):
        return nc.declare_dram_parameter(name, list(shape), dt, isOutput=False)

    def outp(name, shape, dt):
        return nc.declare_dram_parameter(name, list(shape), dt, isOutput=True)

    ident_d = inp("ident", [128, 128], f32)
    identr_d = inp("identr", [128, 128], f32r)
    x0_d = inp("x0s", [NCT, 128, X0F], f32r)
    xs_d = {l["li"]: inp(f"x{l['li']}s", [NCT, 128, l["Nidx"]], f32) for l in LV}
    w_d = [inp(f"w{i}T", [128, NCT * 9 * NCT * 128], f32r) for i in range(4)]
    ow_d = {l["li"]: inp(f"ow{l['li']}T", [128, NCT * 9 * 18], f32r) for l in LV}
    b_d = [inp(f"b{i}", [128, NCT], f32) for i in range(4)]
    ob_d = {l["li"]: inp(f"ob{l['li']}", [18, 1], f32) for l in LV}
    pyb_d = {l["li"]: inp(f"pyb{l['li']}", [128, l["NK"]], f32) for l in LV}
    pxb_d = {l["li"]: inp(f"pxb{l['li']}", [128, l["NK"]], f32) for l in LV}
    selm_d = inp("selm", [128, 16], f32)

    y_d = [outp("y0", [NCT, 128, R0 * W0], f32)] + [
        outp(f"y{l['li']}", [NCT, 128, l["Nidx"]], f32) for l in LV]

    gsrc_d = {l["li"]: nc.dram_tensor(f"gsrc{l['li']}", [l["NPX"], C], bf16)
              for l in LV}
    ag_d = {}
    for l in LV:
        li = l["li"]
        n_in = l["Nidx"] if l["full_ag"] else 2 * l["P"] * l["W"]
        ag_d[li] = dict(
            n_in=n_in,
            cl_in=nc.dram_tensor(f"agincl{li}", [n_in, C], bf16),
            cl_out=nc.dram_tensor(f"agoutcl{li}", [4, n_in, C], bf16),
            cm_in=nc.dram_tensor(f"agincm{li}", [2, NCT, 128, l["W"]], f32),
            cm_out=nc.dram_tensor(f"agoutcm{li}", [4, 2, NCT, 128, l["W"]], f32),
        )

    GROUPS = [[0, 1, 2, 3], [4, 5, 6, 7]]

    with tile.TileContext(nc) as tc:
        with (
            tc.tile_pool(name="cpool", bufs=1) as cpool,
            tc.tile_pool(name="sb", bufs=1) as sb,
            tc.tile_pool(name="ps", bufs=1, space="PSUM") as psp,
        ):
            nc.gpsimd.load_library(library_config.mlp)

            ident = cpool.tile([128, 128], f32, name="ident_sb")
            nc.sync.dma_start(ident[:], ident_d[:])
            identr = cpool.tile([128, 128], f32r, name="identr_sb")
            nc.sync.dma_start(identr[:], identr_d[:])
            selm = cpool.tile([128, 16], f32, name="selm_sb")
            nc.sync.dma_start(selm[:], selm_d[:])

            # ================= level 0 =================
            x0sb = sb.tile([128, NCT, X0F], f32r, name="x0sb", tag="x0sb")
            for ct in range(NCT):
                nc.sync.dma_start(x0sb[:, ct, :], x0_d[ct])
            w0sb = sb.tile([128, NCT * 9 * NCT * 128], f32r, name="w0sb",
                           tag="wsb")
            nc.sync.dma_start(w0sb[:], w_d[0][:])
            b0sb = cpool.tile([128, NCT], f32, name="b0sb")
            nc.sync.dma_start(b0sb[:], b_d[0][:])

            # pooled output of level 0 (input to level 1), plus staging for y0
            PLW = (R0 // 2) * (W0 // 2)
            pooled = sb.tile([128, NCT, PLW], f32, name="pooled0", tag="pooled")
            RPG = max(2, 512 // W0)
            for ot in range(NCT):
                for rg in range(R0 // RPG):
                    acc = psp.tile([128, RPG * W0], f32, name="l0acc",
                                   tag="acc", bufs=4)
                    for ct in range(NCT):
                        for k in range(9):
                            ky, kx = k // 3, k % 3
                            wsl = w0sb[:, ((ct * 9 + k) * NCT + ot) * 128:
                                       ((ct * 9 + k) * NCT + ot + 1) * 128]
                            base = x0sb[:, ct, :]
                            rhs = bass.AP(
                                base.tensor,
                                base.offset + (rg * RPG + ky) * W0p + kx,
                                [base.ap[0], [W0p, RPG], [1, W0]])
                            nc.tensor.matmul(acc[:], wsl, rhs,
                                             start=(ct == 0 and k == 0),
                                             stop=(ct == NCT - 1 and k == 8))
                    sl = slice(rg * RPG * W0, (rg + 1) * RPG * W0)
                    stage = sb.tile([128, RPG * W0], f32, name="l0st",
                                    tag="l0st", bufs=3)
                    nc.any.tensor_scalar_add(stage[:], acc[:],
                                             b0sb[:, ot: ot + 1])
                    nc.sync.dma_start(y_d[0][ot][:, sl], stage[:])
                    # 2x2 max pool from the biased staging tile (SBUF)
                    m1p = sb.tile([128, (RPG // 2) * (W0 // 2)], f32,
                                  name="m1p", tag="pool1")
                    m2p = sb.tile([128, (RPG // 2) * (W0 // 2)], f32,
                                  name="m2p", tag="pool2")
                    def pacc(o, _a=stage):
                        return bass.AP(_a.tensor, _a.offset + o,
                                       [_a.ap[0], [2 * W0, RPG // 2],
                                        [2, W0 // 2]])
                    nc.vector.tensor_tensor(m1p[:], pacc(0), pacc(1), AT.max)
                    nc.vector.tensor_tensor(m2p[:], pacc(W0), pacc(W0 + 1),
                                            AT.max)
                    psl = slice(rg * (RPG // 2) * (W0 // 2),
                                (rg + 1) * (RPG // 2) * (W0 // 2))
                    nc.vector.tensor_tensor(pooled[:, ot, psl], m1p[:],
                                            m2p[:], AT.max)

            W_prev = W0

            # ================= levels 1..3 =================
            for l in LV:
                li, W, R, P, PX = l["li"], l["W"], l["R"], l["P"], l["PX"]
                Nidx, Nidx_pad, Nt = l["Nidx"], l["Nidx_pad"], l["Nt"]
                Rg, Wg, NK = l["Rg"], l["Wg"], l["NK"]
                full_ag, NPXs = l["full_ag"], l["NPX"]
                npos = min(128, Nidx)
                ag = ag_d[li]

                wsb = sb.tile([128, NCT * 9 * NCT * 128], f32r,
                              name=f"wsb{li}", tag="wsb")
                nc.sync.dma_start(wsb[:], w_d[li][:])
                owsb = sb.tile([128, NCT * 9 * 18], f32r, name=f"owsb{li}",
                               tag="owsb")
                nc.sync.dma_start(owsb[:], ow_d[li][:])
                bsb = cpool.tile([128, NCT], f32, name=f"bsb{li}")
                nc.sync.dma_start(bsb[:], b_d[li][:])
                obsb = cpool.tile([18, 1], f32, name=f"obsb{li}")
                nc.sync.dma_start(obsb[:], ob_d[li][:])
                xsb = sb.tile([128, NCT, Nidx], f32, name=f"xsb{li}", tag="xsb")
                for ct in range(NCT):
                    nc.sync.dma_start(xsb[:, ct, :], xs_d[li][ct])
                pyb = sb.tile([128, NK], f32, name=f"pybsb{li}", tag="pyb")
                nc.sync.dma_start(pyb[:], pyb_d[li][:])
                pxb = sb.tile([128, NK], f32, name=f"pxbsb{li}", tag="pxb")
                nc.sync.dma_start(pxb[:], pxb_d[li][:])

                # ---- bum = pooled_prev + x (cm f32r)
                bum = sb.tile([128, NCT, Nidx], f32r, name=f"bum{li}", tag="bum")
                for ct in range(NCT):
                    nc.vector.tensor_tensor(bum[:, ct, :], pooled[:, ct, :],
                                            xsb[:, ct, :], AT.add)

                # ---- bum -> channel-last bf16 tiles
                cl = sb.tile([128, Nt, C], bf16, name=f"cl{li}", tag="cl")
                for t in range(Nt):
                    np_t = min(128, max(0, Nidx - t * 128))
                    pst = psp.tile([128, 256], f32r, name="clps", tag="tr",
                                   bufs=2)
                    for ct in range(NCT):
                        nc.tensor.transpose(
                            pst[:np_t, ct * 128: ct * 128 + 128],
                            bum[:, ct, t * 128: t * 128 + np_t],
                            identr[:])
                    nc.any.tensor_copy(cl[:np_t, t, :], pst[:np_t, :])

                # ---- exchange inputs
                if full_ag:
                    dst = bass.AP(ag["cl_in"], 0, [[C, Nidx], [1, C]])
                    nc.sync.dma_start(dst, cl[:npos, 0, :])
                else:
                    tP = (P * W) // 128
                    for t in range(tP):
                        dst = bass.AP(ag["cl_in"], t * 128 * C,
                                      [[C, 128], [1, C]])
                        nc.sync.dma_start(dst, cl[:, t, :])
                        dst = bass.AP(ag["cl_in"],
                                      (P * W + t * 128) * C,
                                      [[C, 128], [1, C]])
                        nc.sync.dma_start(dst, cl[:, Nt - tP + t, :])
                for ct in range(NCT):
                    nc.sync.dma_start(ag["cm_in"][0, ct],
                                      bum[:, ct, 0:W].bitcast(f32))
                    nc.sync.dma_start(ag["cm_in"][1, ct],
                                      bum[:, ct, (R - 1) * W: R * W].bitcast(f32))
                if use_coll:
                    nc.gpsimd.collective_compute(
                        "AllGather", AT.bypass, replica_groups=GROUPS,
                        ins=[ag["cl_in"][:]], outs=[ag["cl_out"][:]])
                    nc.gpsimd.collective_compute(
                        "AllGather", AT.bypass, replica_groups=GROUPS,
                        ins=[ag["cm_in"][:]], outs=[ag["cm_out"][:]])
                else:
                    nc.sync.dma_start(ag["cl_out"][0], ag["cl_in"][:])
                    nc.sync.dma_start(ag["cm_out"][0], ag["cm_in"][:])

                # ---- zero-fill gather source
                zb = sb.tile([128, 1024], bf16, name=f"zb{li}", tag="zb")
                nc.vector.memset(zb[:], 0.0)
                total = NPXs * C
                CH = 128 * 1024
                for off in range(0, total, CH):
                    n = min(CH, total - off)
                    nper = n // 128
                    nc.sync.dma_start(
                        bass.AP(gsrc_d[li], off, [[nper, 128], [1, nper]]),
                        zb[:, 0:nper])

                def wr_rows(src_ap, row0, nrows, _li=li, _Wg=Wg, _PX=PX, _W=W):
                    dst = bass.AP(gsrc_d[_li], (row0 * _Wg + _PX) * C,
                                  [[_Wg * C, nrows], [C, _W], [1, C]])
                    nc.sync.dma_start(dst, src_ap)

                if Nt == 1:
                    wr_rows(cl[:npos, 0, :], P, R)
                else:
                    rpt = 128 // W
                    for t in range(Nt):
                        wr_rows(cl[:, t, :], P + t * rpt, rpt)

                # ---- halo assembly (masked sums of AllGather blocks)
                n_in = ag["n_in"]
                npb = min(128, n_in)
                nbt = max(1, n_in // 128)
                hb = sb.tile([128, 4 * nbt, C], bf16, name=f"hb{li}", tag="hb")
                for j in range(4):
                    for t in range(nbt):
                        src = bass.AP(ag["cl_out"],
                                      (j * n_in + t * npb) * C,
                                      [[C, npb], [1, C]])
                        nc.sync.dma_start(hb[:npb, j * nbt + t, :], src)

                ht = sb.tile([128, nbt, C], bf16, name=f"ht{li}", tag="ht")
                hbo = sb.tile([128, nbt, C], bf16, name=f"hbo{li}", tag="hbo")

                def masked_sum(dst, np_, blocks, mask0):
                    nc.vector.tensor_scalar_mul(
                        dst, hb[:np_, blocks[0], :], selm[:np_, mask0: mask0 + 1])
                    for j in range(1, 4):
                        nc.vector.scalar_tensor_tensor(
                            dst, hb[:np_, blocks[j], :],
                            selm[:np_, mask0 + j: mask0 + j + 1],
                            dst, AT.mult, AT.add)

                if not full_ag:
                    tP = (P * W) // 128
                    rpt = 128 // W
                    for t in range(tP):
                        masked_sum(ht[:, t, :], 128,
                                   [j * 2 * tP + tP + t for j in range(4)], 0)
                        masked_sum(hbo[:, t, :], 128,
                                   [j * 2 * tP + t for j in range(4)], 4)
                    for t in range(tP):
                        wr_rows(ht[:, t, :], t * rpt, rpt)
                        wr_rows(hbo[:, t, :], P + R + t * rpt, rpt)
                else:
                    # hb block j = rank j's whole R-row slice
                    masked_sum(ht[:npb, 0, :], npb, [0, 1, 2, 3], 8)   # q-2
                    wr_rows(ht[:npb, 0, :], 0, R)
                    masked_sum(ht[:npb, 0, :], npb, [0, 1, 2, 3], 0)   # q-1
                    wr_rows(ht[:npb, 0, :], R, R)
                    masked_sum(hbo[:npb, 0, :], npb, [0, 1, 2, 3], 4)  # q+1
                    wr_rows(hbo[:npb, 0, :], P + R, R)
                    masked_sum(hbo[:npb, 0, :], npb, [0, 1, 2, 3], 12)  # q+2
                    wr_rows(hbo[:npb, 0, :], P + 2 * R, R)

                # ---- cm halo rows for the offset conv
                hcm = sb.tile([128, 4 * 2 * NCT, W], f32, name=f"hcm{li}",
                              tag="hcm")
                for j in range(4):
                    for s in range(2):
                        for ct in range(NCT):
                            nc.sync.dma_start(
                                hcm[:, (j * 2 + s) * NCT + ct, :],
                                ag["cm_out"][j, s, ct])
                hrt = sb.tile([128, NCT, W], f32, name=f"hrt{li}", tag="hrt")
                hrb = sb.tile([128, NCT, W], f32, name=f"hrb{li}", tag="hrb")
                for ct in range(NCT):
                    nc.vector.tensor_scalar_mul(
                        hrt[:, ct, :], hcm[:, 1 * NCT + ct - NCT + NCT, :]
                        if False else hcm[:, (0 * 2 + 1) * NCT + ct, :],
                        selm[:, 0:1])
                    for j in range(1, 4):
                        nc.vector.scalar_tensor_tensor(
                            hrt[:, ct, :], hcm[:, (j * 2 + 1) * NCT + ct, :],
                            selm[:, j: j + 1], hrt[:, ct, :], AT.mult, AT.add)
                    nc.vector.tensor_scalar_mul(
                        hrb[:, ct, :], hcm[:, (0 * 2 + 0) * NCT + ct, :],
                        selm[:, 4:5])
                    for j in range(1, 4):
                        nc.vector.scalar_tensor_tensor(
                            hrb[:, ct, :], hcm[:, (j * 2 + 0) * NCT + ct, :],
                            selm[:, 4 + j: 5 + j], hrb[:, ct, :], AT.mult,
                            AT.add)

                # ---- offset conv on zero-padded bum
                Wp = W + 2
                bumpad = sb.tile([128, NCT, (R + 2) * Wp], f32r,
                                 name=f"bumpad{li}", tag="bumpad")
                zf = sb.tile([128, 64], f32, name=f"zf{li}", tag="zf")
                nc.vector.memset(zf[:], 0.0)
                for ct in range(NCT):
                    base = bumpad[:, ct, :]
                    # zero left/right pad columns (rows fully covered below)
                    nc.vector.tensor_copy(
                        bass.AP(base.tensor, base.offset,
                                [base.ap[0], [Wp, R + 2], [1, 1]]),
                        zf[:, 0: R + 2].rearrange("p (a b) -> p a b", b=1))
                    nc.vector.tensor_copy(
                        bass.AP(base.tensor, base.offset + W + 1,
                                [base.ap[0], [Wp, R + 2], [1, 1]]),
                        zf[:, 0: R + 2].rearrange("p (a b) -> p a b", b=1))
                    nc.vector.tensor_copy(
                        bass.AP(base.tensor, base.offset + Wp + 1,
                                [base.ap[0], [Wp, R], [1, W]]),
                        bum[:, ct, :])
                    nc.vector.tensor_copy(
                        bass.AP(base.tensor, base.offset + 1,
                                [base.ap[0], [1, W]]),
                        hrt[:, ct, :])
                    nc.vector.tensor_copy(
                        bass.AP(base.tensor, base.offset + (R + 1) * Wp + 1,
                                [base.ap[0], [1, W]]),
                        hrb[:, ct, :])

                offs = sb.tile([18, Nidx], f32, name=f"offs{li}", tag="offs")
                RPB = max(1, 512 // W)
                for rb in range(0, R, RPB):
                    nr = min(RPB, R - rb)
                    oacc = psp.tile([18, 512], f32, name="oacc", tag="tr",
                                    bufs=2)
                    for ct in range(NCT):
                        for k in range(9):
                            ky, kx = k // 3, k % 3
                            wsl = owsb[:, (ct * 9 + k) * 18:
                                       (ct * 9 + k + 1) * 18]
                            base = bumpad[:, ct, :]
                            rhs = bass.AP(base.tensor,
                                          base.offset + (rb + ky) * Wp + kx,
                                          [base.ap[0], [Wp, nr], [1, W]])
                            nc.tensor.matmul(oacc[:, 0: nr * W], wsl, rhs,
                                             start=(ct == 0 and k == 0),
                                             stop=(ct == NCT - 1 and k == 8))
                    nc.any.tensor_scalar_add(offs[:, rb * W: (rb + nr) * W],
                                             oacc[:, 0: nr * W], obsb[:])

                # ---- transpose offsets -> OFF_T [128, Nt*18]
                offT = sb.tile([128, Nt * 18], f32, name=f"offT{li}",
                               tag="offT")
                if npos < 128:
                    nc.vector.memset(offT[:], 0.0)
                for t in range(Nt):
                    np_t = min(128, max(0, Nidx - t * 128))
                    opst = psp.tile([128, 512], f32, name="opst", tag="tr",
                                    bufs=2)
                    nc.tensor.transpose(opst[:np_t, 0:18],
                                        offs[:, t * 128: t * 128 + np_t],
                                        ident[0:18, 0:18])
                    nc.any.tensor_copy(offT[:np_t, t * 18: (t + 1) * 18],
                                       opst[:np_t, 0:18])

                # ---- index & weight computation (position-major)
                def offv(par):
                    return bass.AP(offT.tensor, offT.offset + par,
                                   [offT.ap[0], [18, Nt], [2, 9]])

                py = sb.tile([128, NK], f32, name=f"py{li}", tag="py")
                px = sb.tile([128, NK], f32, name=f"px{li}", tag="px")
                nc.vector.tensor_tensor(py[:], pyb[:], offv(0), AT.add)
                nc.vector.tensor_tensor(px[:], pxb[:], offv(1), AT.add)
                nc.vector.tensor_scalar(py[:], py[:], 0.0,
                                        float(Rg - 2) + 0.99, AT.max, AT.min)
                nc.vector.tensor_scalar(px[:], px[:], 0.0,
                                        float(Wg - 2) + 0.99, AT.max, AT.min)
                yi = sb.tile([128, NK], i32, name=f"yi{li}", tag="yi")
                yf = sb.tile([128, NK], f32, name=f"yf{li}", tag="yf")
                gtt = sb.tile([128, NK], f32, name=f"gtt{li}", tag="gtt")
                dy = sb.tile([128, NK], f32, name=f"dy{li}", tag="dy")
                dx = sb.tile([128, NK], f32, name=f"dx{li}", tag="dx")
                y0f = sb.tile([128, NK], f32, name=f"y0f{li}", tag="y0f")
                x0f = sb.tile([128, NK], f32, name=f"x0f{li}", tag="x0f")
                for (pv, d, of) in ((py, dy, y0f), (px, dx, x0f)):
                    nc.vector.tensor_copy(yi[:], pv[:])
                    nc.vector.tensor_copy(yf[:], yi[:])
                    nc.vector.tensor_tensor(gtt[:], yf[:], pv[:], AT.is_gt)
                    nc.vector.tensor_tensor(of[:], yf[:], gtt[:], AT.subtract)
                    nc.vector.tensor_tensor(d[:], pv[:], of[:], AT.subtract)

                wc = sb.tile([128, NK, 4], f32, name=f"wc{li}", tag="wc")
                wy0 = sb.tile([128, NK], f32, name=f"wy0{li}", tag="wy0")
                wx0 = sb.tile([128, NK], f32, name=f"wx0{li}", tag="wx0")
                nc.vector.tensor_scalar(wy0[:], dy[:], -1.0, 1.0, AT.mult,
                                        AT.add)
                nc.vector.tensor_scalar(wx0[:], dx[:], -1.0, 1.0, AT.mult,
                                        AT.add)

                def wcv(cr):
                    return bass.AP(wc.tensor, wc.offset + cr,
                                   [wc.ap[0], [4, NK]])
                nc.vector.tensor_tensor(wcv(0), wy0[:], wx0[:], AT.mult)
                nc.vector.tensor_tensor(wcv(1), wy0[:], dx[:], AT.mult)
                nc.vector.tensor_tensor(wcv(2), dy[:], wx0[:], AT.mult)
                nc.vector.tensor_tensor(wcv(3), dy[:], dx[:], AT.mult)

                # idxf layout: [128, (k, y, t)] col = k*2*Nt + y*Nt + t
                idxf = sb.tile([128, 9, 2, Nt], f32, name=f"idxf{li}",
                               tag="idxf")
                for y in range(2):
                    dsty = bass.AP(idxf.tensor, idxf.offset + y * Nt,
                                   [idxf.ap[0], [1, Nt], [2 * Nt, 9]])
                    if y == 0:
                        nc.vector.scalar_tensor_tensor(
                            dsty, y0f[:], float(Wg), x0f[:], AT.mult, AT.add)
                    else:
                        srcy = bass.AP(idxf.tensor, idxf.offset,
                                       [idxf.ap[0], [1, Nt], [2 * Nt, 9]])
                        nc.vector.tensor_scalar_add(dsty, srcy, float(Wg))
                idx32 = sb.tile([128, 18 * Nt], i32, name=f"idx32{li}",
                                tag="idx32")
                nc.vector.tensor_copy(
                    idx32[:], idxf[:].rearrange("p a b c -> p (a b c)"))
                idx16s = sb.tile([128, 18 * Nt], i16, name=f"idx16s{li}",
                                 tag="idx16s")
                nc.vector.tensor_copy(idx16s[:], idx32[:])

                # ---- reorder idx into gather layout [16-wrap, replicated]
                NS = 2 * Nidx_pad // 16
                idxarr = sb.tile([128, 9, NS], i16, name=f"idxarr{li}",
                                 tag="idxarr")
                for a in range(8):
                    srcv = idx16s[16 * a: 16 * a + 16, :].rearrange(
                        "p (k yt) -> p k yt", k=9)
                    dbase = idxarr[0:16, :, :]
                    dst = bass.AP(dbase.tensor, dbase.offset + a,
                                  [dbase.ap[0], [NS, 9], [8, 2 * Nt]])
                    nc.sync.dma_start(dst, srcv)
                for gix in range(1, 8):
                    nc.sync.dma_start(idxarr[16 * gix: 16 * gix + 16, :, :],
                                      idxarr[0:16, :, :])

                # ---- per-tap gather / combine / transpose / matmul
                NBLK = (Nidx_pad + 511) // 512
                BLKN = min(512, Nidx_pad)
                daccs = [[psp.tile([128, BLKN], f32, name=f"dacc{ot}_{bk}",
                                   tag="acc", bufs=4)
                          for bk in range(NBLK)] for ot in range(NCT)]
                gwin = bass.AP(gsrc_d[li], 0,
                               [[C, NPXs - 1], [1, 2 * C]])
                for k in range(9):
                    g = sb.tile([128, 2 * Nt, 2 * C], bf16, name=f"g{li}",
                                tag="gath", bufs=2)
                    if use_gather:
                        CS = 512  # slots per dma_gather call
                        nch = (2 * Nidx_pad + CS - 1) // CS
                        for c in range(nch):
                            ns = min(CS, 2 * Nidx_pad - c * CS)
                            nc.gpsimd.dma_gather(
                                g[:, c * (CS // 128): c * (CS // 128) +
                                  ns // 128, :],
                                gwin,
                                idxarr[:, k, c * (CS // 16): c * (CS // 16) +
                                       ns // 16],
                                ns, ns, 2 * C, elem_step=C)
                    else:
                        nc.vector.memset(g[:], 0.0)
                    stap = sb.tile([128, NCT, Nidx_pad], f32r,
                                   name=f"stap{li}", tag="stap", bufs=2)
                    strp = [psp.tile([128, Nt * 128], f32, name=f"strp{ct}",
                                     tag="tr", bufs=2)
                            for ct in range(NCT)]
                    for t in range(Nt):
                        stt = sb.tile([128, C], f32, name="stt", tag="stt",
                                      bufs=3)
                        w4 = wc[:, t * 9 + k, :]
                        nc.vector.tensor_scalar_mul(
                            stt[:], g[:, t, 0:C], w4[:, 0:1])
                        nc.vector.scalar_tensor_tensor(
                            stt[:], g[:, t, C: 2 * C], w4[:, 1:2], stt[:],
                            AT.mult, AT.add)
                        nc.vector.scalar_tensor_tensor(
                            stt[:], g[:, Nt + t, 0:C], w4[:, 2:3], stt[:],
                            AT.mult, AT.add)
                        nc.vector.scalar_tensor_tensor(
                            stt[:], g[:, Nt + t, C: 2 * C], w4[:, 3:4], stt[:],
                            AT.mult, AT.add)
                        for ct in range(NCT):
                            nc.tensor.transpose(
                                strp[ct][:, t * 128: (t + 1) * 128],
                                stt[:, ct * 128: (ct + 1) * 128], ident[:])
                    for ct in range(NCT):
                        nc.any.tensor_copy(stap[:, ct, :], strp[ct][:])
                    for ot in range(NCT):
                        for bk in range(NBLK):
                            for ct in range(NCT):
                                wsl = wsb[:, ((ct * 9 + k) * NCT + ot) * 128:
                                          ((ct * 9 + k) * NCT + ot + 1) * 128]
                                nc.tensor.matmul(
                                    daccs[ot][bk][:], wsl,
                                    stap[:, ct, bk * 512: bk * 512 + BLKN],
                                    start=(k == 0 and ct == 0),
                                    stop=(k == 8 and ct == NCT - 1))

                # ---- epilogue: bias+store via staging; pool for next level
                if li < 3:
                    PLWn = (R // 2) * (W // 2)
                    pooled_n = sb.tile([128, NCT, PLWn], f32,
                                       name=f"pooled{li}", tag="pooled")
                for ot in range(NCT):
                    for bk in range(NBLK):
                        n = min(BLKN, Nidx - bk * 512)
                        if n <= 0:
                            continue
                        stage = sb.tile([128, BLKN], f32, name="dst_l",
                                        tag="l0st", bufs=3)
                        nc.any.tensor_scalar_add(
                            stage[:, 0:n], daccs[ot][bk][:, 0:n],
                            bsb[:, ot: ot + 1])
                        nc.sync.dma_start(
                            y_d[li][ot][:, bk * 512: bk * 512 + n],
                            stage[:, 0:n])
                        if li < 3:
                            nrows = n // W
                            m1p = sb.tile([128, (nrows // 2) * (W // 2)], f32,
                                          name="m1d", tag="pool1")
                            m2p = sb.tile([128, (nrows // 2) * (W // 2)], f32,
                                          name="m2d", tag="pool2")
                            def pacc(o, _a=stage, _nr=nrows):
                                return bass.AP(_a.tensor, _a.offset + o,
                                               [_a.ap[0], [2 * W, _nr // 2],
                                                [2, W // 2]])
                            nc.vector.tensor_tensor(m1p[:], pacc(0), pacc(1),
                                                    AT.max)
                            nc.vector.tensor_tensor(m2p[:], pacc(W),
                                                    pacc(W + 1), AT.max)
                            npl = (nrows // 2) * (W // 2)
                            p0 = (bk * (BLKN // W) // 2) * (W // 2)
                            nc.vector.tensor_tensor(
                                pooled_n[:, ot, p0: p0 + npl], m1p[:],
                                m2p[:], AT.max)

                if li < 3:
                    pooled = pooled_n
                W_prev = W

    nc.compile()
    return nc


# ------------------------------------------------------------------- host

def host_prep(inputs, H0=128):
    LV = level_params(H0)
    R0, W0 = H0 // 4, H0
    W0p = W0 + 2
    xs = {i: np.asarray(inputs[f"x{i}"], np.float32) for i in range(4)}
    ws = {i: np.asarray(inputs[f"w{i}"], np.float32) for i in range(4)}
    bs = {i: np.asarray(inputs[f"b{i}"], np.float32) for i in range(4)}
    ows = {i: np.asarray(inputs[f"ow{i}"], np.float32) for i in (1, 2, 3)}
    obs = {i: np.asarray(inputs[f"ob{i}"], np.float32) for i in (1, 2, 3)}

    def wT_pack(w):
        out = np.zeros((128, 2 * 9 * 2 * 128), np.float32)
        for ct in range(2):
            for k in range(9):
                for ot in range(2):
                    blk = w[ot * 128:(ot + 1) * 128,
                            ct * 128:(ct + 1) * 128, k // 3, k % 3]
                    out[:, ((ct * 9 + k) * 2 + ot) * 128:
                        ((ct * 9 + k) * 2 + ot + 1) * 128] = blk.T
        return out

    def owT_pack(w):
        out = np.zeros((128, 2 * 9 * 18), np.float32)
        for ct in range(2):
            for k in range(9):
                blk = w[:, ct * 128:(ct + 1) * 128, k // 3, k % 3]
                out[:, (ct * 9 + k) * 18: (ct * 9 + k + 1) * 18] = blk.T
        return out

    wTs = {i: wT_pack(ws[i]) for i in range(4)}
    owTs = {i: owT_pack(ows[i]) for i in (1, 2, 3)}
    ident = np.eye(128, dtype=np.float32)

    pybs, pxbs = {}, {}
    for l in LV:
        NK, Nt, W, P, PX = l["NK"], l["Nt"], l["W"], l["P"], l["PX"]
        pyb = np.full((128, NK), float(P), np.float32)
        pxb = np.full((128, NK), float(PX), np.float32)
        for t in range(Nt):
            for k in range(9):
                ky, kx = k // 3, k % 3
                for p in range(128):
                    i = t * 128 + p
                    if i >= l["Nidx"]:
                        continue
                    pyb[p, t * 9 + k] = (i // W) + ky - 1 + P
                    pxb[p, t * 9 + k] = (i % W) + kx - 1 + PX
        pybs[l["li"]], pxbs[l["li"]] = pyb, pxb

    x0p = np.pad(xs[0], [(0, 0), (0, 0), (1, 1), (1, 1)])

    in_maps = []
    for core in range(N_CORES):
        n, q = core // 4, core % 4
        selm = np.zeros((128, 16), np.float32)
        for j in range(4):
            if j == q - 1:
                selm[:, 0 + j] = 1.0
            if j == q + 1:
                selm[:, 4 + j] = 1.0
            if j == q - 2:
                selm[:, 8 + j] = 1.0
            if j == q + 2:
                selm[:, 12 + j] = 1.0
        m = dict(ident=ident, identr=ident, selm=selm)
        sl = x0p[n, :, q * R0: q * R0 + R0 + 2, :]
        m["x0s"] = np.ascontiguousarray(
            sl.reshape(2, 128, (R0 + 2) * W0p))
        for l in LV:
            li, R, W = l["li"], l["R"], l["W"]
            xsl = xs[li][n, :, q * R: (q + 1) * R, :]
            m[f"x{li}s"] = np.ascontiguousarray(xsl.reshape(2, 128, R * W))
            m[f"ow{li}T"] = owTs[li]
            m[f"ob{li}"] = obs[li].reshape(18, 1)
            m[f"pyb{li}"] = pybs[li]
            m[f"pxb{li}"] = pxbs[li]
        for i in range(4):
            m[f"w{i}T"] = wTs[i]
            m[f"b{i}"] = np.ascontiguousarray(
                np.stack([bs[i][:128], bs[i][128:]], axis=1))
        in_maps.append(m)
    return in_maps


def host_assemble(results, H0=128):
    LV = level_params(H0)
    R0, W0 = H0 // 4, H0
    outs = []
    shapes = [("y0", R0, W0)] + [(f"y{l['li']}", l["R"], l["W"]) for l in LV]
    for name, Rr, Wr in shapes:
        full = np.zeros((2, C, Rr * 4, Wr), np.float32)
        for core in range(N_CORES):
            n, q = core // 4, core % 4
            v = np.asarray(results[core][name]).reshape(C, Rr, Wr)
            full[n, :, q * Rr: (q + 1) * Rr, :] = v
        outs.append(full)
    return tuple(outs)


_NC_CACHE = {}


def kernel(**inputs):
    from concourse.bass_utils import run_bass_kernel_spmd
    H0 = int(np.asarray(inputs["x0"]).shape[2])
    if H0 not in _NC_CACHE:
        _NC_CACHE[H0] = build_nc(H0)
    nc = _NC_CACHE[H0]
    in_maps = host_prep(inputs, H0)
    res = run_bass_kernel_spmd(nc, in_maps, list(range(N_CORES)))
    return host_assemble(res.results, H0)
